# revision 5
# baseline (speedup 1.0000x reference)
"""CenterLoss kernel for Trainium2 (Bass/Tile), 8-core data-parallel.

loss = sum_i ||x_i - centers[labels_i]||^2
  x: (65536, 512) f32, labels: (65536,) int, centers: (512, 512) f32

Per-core plan (8192 rows each), using the expansion
  loss = sum x^2 - 2*sum_{c,d} S[c,d]*centers[c,d] + sum_c count_c*||C_c||^2
with S = onehot(labels)^T @ x computed on the PE via one-hot matmuls
(exactly representable in fp8). The third (histogram) term depends only on
labels+centers and is computed exactly on the host.
  - x streamed HBM->SBUF with an in-flight f32->fp8e4 cast (SWDGE), 4
    consecutive rows per partition so each DMA descriptor reads 8KB; the
    last 512 rows are split into two 256-row units so the final PE burst
    after the last DMA byte is short
  - DVE builds the one-hot tile: is_equal(iota_row, label_p); the iota
    pattern is class-permuted (class 4m+c at column c*128+m) so the S psum
    layout lines up with an 8KB-descriptor centers load
  - PE: per 256-row group, 4 DoubleRow matmuls accumulate S chunks in PSUM
  - ACT accumulates sum(x^2) per unit
  - tail: 4 per-chunk DVE contractions of S with centers overlap the last
    matmuls, then a tiny f32 matmul against a ones column collapses the
    [128,*] partials to one partition so the output DMA is a single
    descriptor; host sums the per-core partial vector.
"""

import sys

import numpy as np

sys.path.insert(0, "/opt/trn_rl_repo")

N_CORES = 8
B = 65536
D = 512
B_L = B // N_CORES  # 8192 rows per core
NCH = D // 128  # 4 class chunks
# x DMA units: rows per unit; 8KB descriptors (4 rows/partition) for the
# bulk, two 256-row (4KB) units at the end to shorten the tail
UNITS = [512] * 15 + [256, 256]
N_UNITS = len(UNITS)
N_ACC = N_UNITS + NCH  # acc_all columns: per-unit sum(x^2) + per-chunk -2*S.C

assert sum(UNITS) == B_L

_CACHE = {}


def _build():
    """Trace the Bass/Tile program once; returns the compiled Bacc module."""
    if "nc" in _CACHE:
        return _CACHE["nc"]

    import concourse.bacc as bacc
    import concourse.mybir as mybir
    import concourse.tile as tile

    f32 = mybir.dt.float32
    fp8 = mybir.dt.float8e4

    nc = bacc.Bacc("TRN2", debug=False, num_devices=N_CORES)
    x_t = nc.dram_tensor("x", [B_L, D], f32, kind="ExternalInput")
    iota_t = nc.dram_tensor("iota16", [128, D], mybir.dt.float16, kind="ExternalInput")
    labf_t = nc.dram_tensor("labf", [128, B_L // 128], f32, kind="ExternalInput")
    c_t = nc.dram_tensor("centers", [D, D], f32, kind="ExternalInput")
    out_t = nc.dram_tensor("out", [1, N_ACC], f32, kind="ExternalOutput")

    with tile.TileContext(nc) as tc:
        with (
            tc.tile_pool(name="io", bufs=12) as io_pool,
            tc.tile_pool(name="oh", bufs=8) as oh_pool,
            tc.tile_pool(name="psum", bufs=1, space="PSUM") as psum_pool,
            tc.tile_pool(name="misc", bufs=1) as misc_pool,
        ):
            # small inputs first on the HWDGE queue so their transfers land
            # before the x stream saturates the DMA engines
            labf_sb = misc_pool.tile([128, B_L // 128], f32)
            nc.sync.dma_start(labf_sb[:], labf_t.ap())
            iota_sb = misc_pool.tile([128, D], mybir.dt.float16)
            nc.sync.dma_start(iota_sb[:], iota_t.ap())
            # centers: partition p holds rows 4p..4p+3 -> 8KB descriptors;
            # matches the permuted one-hot class order (class 4m+c at S[m,c])
            cent_sb = misc_pool.tile([128, NCH, D], f32)
            nc.sync.dma_start(
                cent_sb[:], c_t.ap().rearrange("(p n) d -> p n d", n=NCH)
            )

            ones_col = misc_pool.tile([128, 1], f32)
            nc.vector.memset(ones_col[:], 1.0)
            acc_all = misc_pool.tile([128, N_ACC], f32)
            junk_dve = misc_pool.tile([128, 1], f32)
            junk_act = misc_pool.tile([128, 1], f32)

            S_all = psum_pool.tile([128, NCH, D], f32, name="S_all")
            S_ps = [S_all[:, c, :] for c in range(NCH)]
            out_ps = psum_pool.tile([128, N_ACC], f32, tag="fin", name="out_ps")

            x_ap = x_t.ap()
            row0 = 0
            col0 = 0
            for k, rows in enumerate(UNITS):
                q = rows // 128
                x_sb = io_pool.tile([128, 4, D], fp8, tag="x")
                nc.gpsimd.dma_start(
                    x_sb[:, 0:q, :],
                    x_ap[row0 : row0 + rows, :].rearrange("(p q) d -> p q d", q=q),
                )
                last_unit = k == N_UNITS - 1
                for j in range(q // 2):
                    oh = oh_pool.tile([128, 2, D], fp8, tag="oh")
                    for u in range(2):
                        t = col0 + 2 * j + u
                        nc.vector.tensor_scalar(
                            out=oh[:, u, :],
                            in0=iota_sb[:],
                            scalar1=labf_sb[:, t : t + 1],
                            scalar2=None,
                            op0=mybir.AluOpType.is_equal,
                        )
                    first = k == 0 and j == 0
                    last = last_unit and j == q // 2 - 1
                    for c in range(NCH):
                        nc.tensor.matmul(
                            S_ps[c],
                            lhsT=oh[:, :, c * 128 : (c + 1) * 128],
                            rhs=x_sb[:, 2 * j : 2 * j + 2, :],
                            start=first,
                            stop=last,
                            perf_mode=mybir.MatmulPerfMode.DoubleRow,
                        )
                        if last:
                            # chunk c of S is complete: contract with centers
                            # while the remaining chunks' matmuls still run
                            nc.vector.scalar_tensor_tensor(
                                out=junk_dve[:].broadcast_to(S_ps[c].shape),
                                in0=S_ps[c],
                                scalar=-2.0,
                                in1=cent_sb[:, c, :],
                                op0=mybir.AluOpType.mult,
                                op1=mybir.AluOpType.mult,
                                accum_out=acc_all[:, N_UNITS + c : N_UNITS + c + 1],
                            )
                # sum(x^2) on ACT, one op per unit
                x_flat = x_sb[:, 0:q, :].rearrange("p q d -> p (q d)")
                nc.scalar.activation(
                    junk_act[:].broadcast_to(x_flat.shape),
                    x_flat,
                    mybir.ActivationFunctionType.Square,
                    accum_out=acc_all[:, k : k + 1],
                )
                row0 += rows
                col0 += q

            # collapse partitions: out_ps[0, k] = sum_p acc_all[p, k], so the
            # result lives on one partition and the out DMA is one descriptor
            nc.tensor.matmul(
                out_ps[0:1, :],
                lhsT=ones_col[:],
                rhs=acc_all[:],
                start=True,
                stop=True,
            )
            out_sb = misc_pool.tile([128, N_ACC], f32)
            nc.vector.tensor_copy(out_sb[0:1, :], out_ps[0:1, :])
            nc.sync.dma_start(out_t.ap(), out_sb[0:1, :])

    nc.compile()
    _CACHE["nc"] = nc
    return nc


def _prep_inputs(x, labels, centers):
    """Shard full inputs into the 8 per-core input maps."""
    x = np.asarray(x, dtype=np.float32)
    labels = np.asarray(labels)
    centers = np.ascontiguousarray(np.asarray(centers, dtype=np.float32))
    # permuted iota: column c*128+m holds class 4m+c, matching the centers
    # SBUF layout [p, n, :] = centers row 4p+n
    cols = np.arange(D)
    iota_vals = (4 * (cols % 128) + (cols // 128)).astype(np.float16)
    iota16 = np.ascontiguousarray(np.tile(iota_vals, (128, 1)))
    in_maps = []
    for cix in range(N_CORES):
        xs = np.ascontiguousarray(x[cix * B_L : (cix + 1) * B_L])
        lab = labels[cix * B_L : (cix + 1) * B_L]
        # labf[p, col0+v] = label of unit-k row q*p+v (q rows/partition)
        labf = np.empty((128, B_L // 128), dtype=np.float32)
        row0 = 0
        col0 = 0
        for rows in UNITS:
            q = rows // 128
            blk = lab[row0 : row0 + rows].reshape(128, q)
            labf[:, col0 : col0 + q] = blk.astype(np.float32)
            row0 += rows
            col0 += q
        labf = np.ascontiguousarray(labf)
        in_maps.append({"x": xs, "iota16": iota16, "labf": labf, "centers": centers})
    return in_maps


def _run(x, labels, centers, trace=False):
    from concourse import bass_utils

    nc = _build()
    in_maps = _prep_inputs(x, labels, centers)
    res = bass_utils.run_bass_kernel_spmd(
        nc, in_maps, core_ids=list(range(N_CORES)), trace=trace
    )
    total = np.float64(0.0)
    for r in res.results:
        total += np.sum(r["out"].astype(np.float64))
    # exact histogram term on host: sum_c count_c * ||C_c||^2
    labels_np = np.asarray(labels).astype(np.int64)
    counts = np.bincount(labels_np, minlength=D).astype(np.float64)
    csq = (np.asarray(centers).astype(np.float64) ** 2).sum(axis=1)
    total += float(counts @ csq)
    return np.array(total, dtype=np.float32), res


def kernel(x, labels, centers):
    out, _ = _run(x, labels, centers, trace=False)
    return out


def kernel_traced(x, labels, centers):
    return _run(x, labels, centers, trace=True)


# revision 6
# speedup vs baseline: 1.1579x; 1.1579x over previous
"""CenterLoss kernel for Trainium2 (Bass/Tile), 8-core data-parallel.

loss = sum_i ||x_i - centers[labels_i]||^2
  x: (65536, 512) f32, labels: (65536,) int, centers: (512, 512) f32

Per-core plan (8192 rows each), using the expansion
  loss = sum x^2 - 2*sum_{c,d} S[c,d]*centers[c,d] + sum_c count_c*||C_c||^2
with S = onehot(labels)^T @ x computed on the PE via one-hot matmuls
(exactly representable in fp8). The third (histogram) term depends only on
labels+centers and is computed exactly on the host.
"""

import sys

import numpy as np

sys.path.insert(0, "/opt/trn_rl_repo")

N_CORES = 8
B = 65536
D = 512
B_L = B // N_CORES  # 8192 rows per core
SUPER = 512  # rows per supertile (x DMA granularity)
N_SUPER = B_L // SUPER  # 16
Q = SUPER // 128  # 4 rows per partition per supertile -> 8KB descriptors
N_TILES = B_L // 128  # 64 label columns
NCH = D // 128  # 4 class chunks

_CACHE = {}


def _build():
    """Trace the Bass/Tile program once; returns the compiled Bacc module."""
    if "nc" in _CACHE:
        return _CACHE["nc"]

    import concourse.bacc as bacc
    import concourse.mybir as mybir
    import concourse.tile as tile

    f32 = mybir.dt.float32
    fp8 = mybir.dt.float8e4

    nc = bacc.Bacc("TRN2", debug=False, num_devices=N_CORES)
    x_t = nc.dram_tensor("x", [B_L, D], f32, kind="ExternalInput")
    iota_t = nc.dram_tensor("iota16", [128, D], mybir.dt.float16, kind="ExternalInput")
    labf_t = nc.dram_tensor("labf", [128, N_TILES], f32, kind="ExternalInput")
    c_t = nc.dram_tensor("centers", [D, D], f32, kind="ExternalInput")
    out_t = nc.dram_tensor("out", [1, N_SUPER + 1], f32, kind="ExternalOutput")

    with tile.TileContext(nc) as tc:
        with (
            tc.tile_pool(name="io", bufs=12) as io_pool,
            tc.tile_pool(name="oh", bufs=8) as oh_pool,
            tc.tile_pool(name="psum", bufs=1, space="PSUM") as psum_pool,
            tc.tile_pool(name="misc", bufs=1) as misc_pool,
        ):
            # small inputs first on the HWDGE queue so their transfers land
            # before the x stream saturates the DMA engines
            labf_sb = misc_pool.tile([128, N_TILES], f32)
            nc.sync.dma_start(labf_sb[:], labf_t.ap())
            iota_sb = misc_pool.tile([128, D], mybir.dt.float16)
            nc.sync.dma_start(iota_sb[:], iota_t.ap())
            cent_sb = misc_pool.tile([128, NCH, D], f32)
            nc.sync.dma_start(
                cent_sb[:], c_t.ap().rearrange("(n p) d -> p n d", p=128)
            )

            ones_col = misc_pool.tile([128, 1], f32)
            nc.vector.memset(ones_col[:], 1.0)
            # cols 0..N_SUPER-1: per-supertile sum(x^2); col N_SUPER: -2*S.C
            acc_all = misc_pool.tile([128, N_SUPER + 1], f32)
            junk_dve = misc_pool.tile([128, 1], f32)
            junk_act = misc_pool.tile([128, 1], f32)
            out_sb = misc_pool.tile([128, N_SUPER + 1], f32)

            S_all = psum_pool.tile([128, NCH, D], f32, name="S_all")
            S_ps = [S_all[:, c, :] for c in range(NCH)]
            out_ps = psum_pool.tile(
                [128, N_SUPER + 1], f32, tag="fin", name="out_ps"
            )

            x_ap = x_t.ap()
            for s in range(N_SUPER):
                x_sb = io_pool.tile([128, Q, D], fp8, tag="x")
                # SWDGE casts f32 -> fp8e4m3 in flight; partition p holds rows
                # 4p..4p+3 of the supertile so each descriptor reads 8KB
                nc.gpsimd.dma_start(
                    x_sb[:],
                    x_ap[s * SUPER : (s + 1) * SUPER, :].rearrange(
                        "(p q) d -> p q d", q=Q
                    ),
                )
                for j in range(Q // 2):
                    oh = oh_pool.tile([128, 2, D], fp8, tag="oh")
                    for u in range(2):
                        t = s * Q + 2 * j + u
                        nc.vector.tensor_scalar(
                            out=oh[:, u, :],
                            in0=iota_sb[:],
                            scalar1=labf_sb[:, t : t + 1],
                            scalar2=None,
                            op0=mybir.AluOpType.is_equal,
                        )
                    first = s == 0 and j == 0
                    last = s == N_SUPER - 1 and j == Q // 2 - 1
                    for c in range(NCH):
                        nc.tensor.matmul(
                            S_ps[c],
                            lhsT=oh[:, :, c * 128 : (c + 1) * 128],
                            rhs=x_sb[:, 2 * j : 2 * j + 2, :],
                            start=first,
                            stop=last,
                            perf_mode=mybir.MatmulPerfMode.DoubleRow,
                        )
                # sum(x^2) on ACT, one op per supertile
                x_flat = x_sb[:].rearrange("p q d -> p (q d)")
                nc.scalar.activation(
                    junk_act[:].broadcast_to(x_flat.shape),
                    x_flat,
                    mybir.ActivationFunctionType.Square,
                    accum_out=acc_all[:, s : s + 1],
                )

            # tail: acc_all[:, -1] = -2*sum_{c,d} S[c,d]*C[c,d] per partition
            S_flat = S_all[:].rearrange("p c d -> p (c d)")
            C_flat = cent_sb[:].rearrange("p c d -> p (c d)")
            nc.vector.scalar_tensor_tensor(
                out=junk_dve[:].broadcast_to(S_flat.shape),
                in0=S_flat,
                scalar=-2.0,
                in1=C_flat,
                op0=mybir.AluOpType.mult,
                op1=mybir.AluOpType.mult,
                accum_out=acc_all[:, N_SUPER : N_SUPER + 1],
            )
            # collapse partitions: out_ps[0, k] = sum_p acc_all[p, k], so the
            # result lives on one partition and the out DMA is one descriptor
            nc.tensor.matmul(
                out_ps[0:1, :],
                lhsT=ones_col[:],
                rhs=acc_all[:],
                start=True,
                stop=True,
            )
            nc.vector.tensor_copy(out_sb[0:1, :], out_ps[0:1, :])
            nc.sync.dma_start(out_t.ap(), out_sb[0:1, :])

    nc.compile()
    _CACHE["nc"] = nc
    return nc


def _prep_inputs(x, labels, centers):
    """Shard full inputs into the 8 per-core input maps."""
    x = np.asarray(x, dtype=np.float32)
    labels = np.asarray(labels)
    centers = np.ascontiguousarray(np.asarray(centers, dtype=np.float32))
    iota16 = np.ascontiguousarray(
        np.tile(np.arange(D, dtype=np.float16), (128, 1))
    )
    in_maps = []
    for c in range(N_CORES):
        xs = np.ascontiguousarray(x[c * B_L : (c + 1) * B_L])
        lab = labels[c * B_L : (c + 1) * B_L]
        # labf[p, s*Q+v] = label of supertile-s row 4p+v (4 rows/partition)
        labf = np.ascontiguousarray(
            lab.reshape(N_SUPER, 128, Q)
            .transpose(1, 0, 2)
            .reshape(128, N_TILES)
            .astype(np.float32)
        )
        in_maps.append({"x": xs, "iota16": iota16, "labf": labf, "centers": centers})
    return in_maps


def _run(x, labels, centers, trace=False):
    from concourse import bass_utils

    nc = _build()
    in_maps = _prep_inputs(x, labels, centers)
    res = bass_utils.run_bass_kernel_spmd(
        nc, in_maps, core_ids=list(range(N_CORES)), trace=trace
    )
    total = np.float64(0.0)
    for r in res.results:
        total += np.sum(r["out"].astype(np.float64))
    # exact histogram term on host: sum_c count_c * ||C_c||^2
    labels_np = np.asarray(labels).astype(np.int64)
    counts = np.bincount(labels_np, minlength=D).astype(np.float64)
    csq = (np.asarray(centers).astype(np.float64) ** 2).sum(axis=1)
    total += float(counts @ csq)
    return np.array(total, dtype=np.float32), res


def kernel(x, labels, centers):
    out, _ = _run(x, labels, centers, trace=False)
    return out


def kernel_traced(x, labels, centers):
    return _run(x, labels, centers, trace=True)
